# revision 1
# baseline (speedup 1.0000x reference)
"""Trainium2 Bass kernel for attention with softmax over the *query* axis.

Reference computation (B=2, N=8192, D=256, fp32):
    Q = x @ Wq.T ; K = x @ Wk.T ; V = x @ Wv.T          # [B, N, D]
    s = Q @ K.T / sqrt(D)                                # [B, N, N]
    attn = softmax(s, axis=1)       # softmax over the QUERY axis
    out = attn @ V                                       # [B, N, D]

Because the softmax normalizes over the query axis, the stats Z[k] =
sum_q exp(s[q,k]) are per-(batch, key) reductions.  Sharding the KEY
axis across cores keeps the softmax entirely local to a core; the
output out[q,:] = sum_k exp(s[q,k])/Z[k] * V[k,:] is then a sum of
per-core partials, which the host adds at gather time.

Algebraic restructuring used on device (per core; batch b, key chunk c):
    A  = Wq.T @ Wk                [D, D]   (so s = x A x.T, scale folded
                                            into the exp() activation)
    B  = A.T @ x_b.T              [D, N]   (shared across the key chunk)
    sT[k, q] = (x_c B)[k, q]               (keys on partitions!)
    E  = exp(sT / sqrt(D))                 (ACT, with accum_out -> Z[k])
    V' = V / Z[k]                          (per-partition scalar)
    outT_partial = V'.T @ E       [D, N]
Values |s/sqrt(D)| < ~3 for these inputs, so exp() without the max
subtraction is numerically safe (softmax is shift-invariant, so the
result is mathematically identical to the reference).

Everything except the fp32 PSUM accumulation runs in bf16; measured
relative error vs the fp32 reference is a few 1e-3.
"""

import functools

import numpy as np

# ---- problem constants (hardcoded per the harness contract) ----
B = 2
N = 8192
D = 256
N_CORES = 8
CORES_PER_BATCH = N_CORES // B
CHUNK = N // CORES_PER_BATCH          # 2048 keys per core
N_SUB = 4                             # sequential key sub-chunks per core
SUB = CHUNK // N_SUB                  # 512 keys per sub-chunk
QS = 1024                             # query super-tile width
SCALE = 1.0 / 16.0                    # 1/sqrt(D)


def _build_program(n=N, chunk=CHUNK, n_sub=N_SUB, qs=QS, n_devices=N_CORES,
                   enable_asserts=False):
    import concourse.bass as bass
    import concourse.tile as tile
    from concourse import bacc, mybir
    from concourse.masks import make_identity

    f32 = mybir.dt.float32
    bf16 = mybir.dt.bfloat16
    ts = bass.ts
    P = 128

    n_qt = n // P                 # x-transpose tiles over queries
    n_kt = chunk // P             # key tiles per core
    kq = n_kt // n_sub            # key tiles per sub-chunk
    nqs = n // qs                 # query super-tiles

    nc = bacc.Bacc("TRN2", target_bir_lowering=False, debug=False,
                   enable_asserts=enable_asserts, num_devices=n_devices)

    xb = nc.dram_tensor("xb", [n, D], f32, kind="ExternalInput").ap()
    xk = nc.dram_tensor("xk", [chunk, D], f32, kind="ExternalInput").ap()
    wq = nc.dram_tensor("wq", [D, D], f32, kind="ExternalInput").ap()
    wk = nc.dram_tensor("wk", [D, D], f32, kind="ExternalInput").ap()
    wv = nc.dram_tensor("wv", [D, D], f32, kind="ExternalInput").ap()
    out_part = nc.dram_tensor("out_part", [n_sub, D, n], f32,
                              kind="ExternalOutput").ap()

    Exp = mybir.ActivationFunctionType.Exp

    with tile.TileContext(nc) as tc:
        with (
            tc.tile_pool(name="const", bufs=1) as const_pool,
            tc.tile_pool(name="proj", bufs=1) as proj_pool,
            tc.tile_pool(name="xkt", bufs=1) as xkt_pool,
            tc.tile_pool(name="vpool", bufs=1) as v_pool,
            tc.tile_pool(name="bpool", bufs=1) as b_pool,
        ):
            ident = const_pool.tile([P, P], f32)
            make_identity(nc, ident[:])

            # ---------------- phase A: projections ----------------
            A_sb = proj_pool.tile([P, 2, D], bf16)     # A[d, d']
            WvT_sb = proj_pool.tile([P, 2, D], bf16)   # Wv.T[d, j]
            xkT_sb = xkt_pool.tile([P, 2, chunk], bf16)  # x_c.T[d, k]
            V_sb = v_pool.tile([P, n_kt, D], bf16)     # V[k, j] (k tiles)
            B_sb = b_pool.tile([P, 2, n], bf16)        # B[d', q]

            with (
                tc.tile_pool(name="wstage", bufs=1) as wstage,
                tc.tile_pool(name="xt", bufs=1) as xt_pool,
                tc.tile_pool(name="dram", bufs=1, space="DRAM") as dram_pool,
                tc.tile_pool(name="psA", bufs=2, space="PSUM") as psA,
                tc.tile_pool(name="psT", bufs=2, space="PSUM") as psT,
            ):
                wq_sb = wstage.tile([P, 2, D], f32)
                wk_sb = wstage.tile([P, 2, D], f32)
                wv_sb = wstage.tile([P, 2, D], f32)
                nc.sync.dma_start(wq_sb[:], wq.rearrange("(c p) d -> p c d", p=P))
                nc.sync.dma_start(wk_sb[:], wk.rearrange("(c p) d -> p c d", p=P))
                nc.sync.dma_start(wv_sb[:], wv.rearrange("(c p) d -> p c d", p=P))

                # A[d, d'] = sum_i Wq[i, d] * Wk[i, d']
                for dh in range(2):
                    aps = psA.tile([P, D], f32, tag="ps")
                    for ic in range(2):
                        nc.tensor.matmul(aps[:], wq_sb[:, ic, ts(dh, P)],
                                         wk_sb[:, ic, :],
                                         start=(ic == 0), stop=(ic == 1))
                    nc.any.tensor_copy(A_sb[:, dh, :], aps[:])

                # Wv.T[d, j] = Wv[j, d] transposed
                for ic in range(2):
                    for dh in range(2):
                        tps = psT.tile([P, P], f32)
                        nc.tensor.transpose(tps[:], wv_sb[:, ic, ts(dh, P)],
                                            ident[:])
                        nc.any.tensor_copy(WvT_sb[:, dh, ts(ic, P)], tps[:])

                # x transposes via cast-DMA (f32 -> bf16 DRAM scratch, SWDGE)
                # followed by XBAR transpose-DMA back into SBUF.  Keeps the
                # transposes entirely off the compute engines.
                RB = min(1024, n)
                KB = min(1024, chunk)
                # one DRAM scratch tile PER CHUNK: tile-granular dep
                # tracking otherwise makes every transpose wait for the
                # LAST cast into a shared scratch tensor (measured: first
                # matmul at 68us ~= cast-chain end, not its own chunk)
                xbf_c = [dram_pool.tile([RB, D], bf16, name=f"xbfc{rc}")
                         for rc in range(n // RB)]
                xkbf_c = [dram_pool.tile([KB, D], bf16, name=f"xkbfc{rc}")
                          for rc in range(chunk // KB)]
                # cast order: chunks consumed earliest go first (query
                # chunk 0, then the key chunks, then the rest)
                nc.gpsimd.dma_start(xbf_c[0][:], xb[ts(0, RB), :])
                for rc in range(chunk // KB):
                    nc.gpsimd.dma_start(xkbf_c[rc][:], xk[ts(rc, KB), :])
                for rc in range(1, n // RB):
                    nc.gpsimd.dma_start(xbf_c[rc][:], xb[ts(rc, RB), :])

                # transposes: qc-outer / dh-inner.  B(q8) needs BOTH dh
                # halves of query chunk q8; the previous dh-outer order put
                # the dh=1 half of chunk 0 ninth in the serial transpose
                # queue, delaying the first B matmul to ~89us.  Key chunk 0
                # follows immediately so pass 1 of sub-chunk 0 can start.
                xT_sb = xt_pool.tile([P, 2, n], bf16)

                def tr_x(qc):
                    for dh in range(2):
                        nc.sync.dma_start(out=xT_sb[:, dh, ts(qc, RB)],
                                          in_=xbf_c[qc][:, ts(dh, P)],
                                          transpose=True)

                def tr_xk(kc):
                    for dh in range(2):
                        nc.sync.dma_start(out=xkT_sb[:, dh, ts(kc, KB)],
                                          in_=xkbf_c[kc][:, ts(dh, P)],
                                          transpose=True)

                tr_x(0)
                for kc in range(chunk // KB):
                    tr_xk(kc)
                for qc in range(1, n // RB):
                    tr_x(qc)

                # V[k, j] = sum_d x_c[k, d] * Wv[j, d]
                for kt in range(n_kt):
                    vps = psA.tile([P, D], f32, tag="ps")
                    for dh in range(2):
                        nc.tensor.matmul(vps[:], xkT_sb[:, dh, ts(kt, P)],
                                         WvT_sb[:, dh, :],
                                         start=(dh == 0), stop=(dh == 1))
                    nc.any.tensor_copy(V_sb[:, kt, :], vps[:])

                # B[d', q] = sum_d A[d, d'] * x_b.T[d, q]
                # (matmul output must stay within one PSUM bank: N <= 512)
                nmm = qs // 512
                for q8 in range(nqs):
                    for dp in range(2):
                        bps = psA.tile([P, qs], f32, tag="ps")
                        for nh in range(nmm):
                            for dh in range(2):
                                nc.tensor.matmul(
                                    bps[:, ts(nh, 512)], A_sb[:, dh, ts(dp, P)],
                                    xT_sb[:, dh, ts(q8 * nmm + nh, 512)],
                                    start=(dh == 0), stop=(dh == 1))
                        nc.any.tensor_copy(B_sb[:, dp, ts(q8, qs)], bps[:])

            # ---------------- main loop over key sub-chunks ----------------
            with (
                tc.tile_pool(name="epool", bufs=2) as e_pool,
                tc.tile_pool(name="zpool", bufs=2) as z_pool,
                tc.tile_pool(name="vp", bufs=2) as vp_pool,
                tc.tile_pool(name="ostage", bufs=3) as o_pool,
                tc.tile_pool(name="psS", bufs=3, space="PSUM") as psS,
                tc.tile_pool(name="psO", bufs=2, space="PSUM") as psO,
            ):
                for sub in range(n_sub):
                    E_t = e_pool.tile([P, kq, n], bf16)
                    Zp = z_pool.tile([P, kq * nqs], f32)

                    # pass 1: scores -> exp -> E (and Z partials)
                    for q8 in range(nqs):
                        for kt in range(kq):
                            ktg = sub * kq + kt
                            sps = psS.tile([P, qs], f32)
                            for nh in range(nmm):
                                for dh in range(2):
                                    nc.tensor.matmul(
                                        sps[:, ts(nh, 512)],
                                        xkT_sb[:, dh, ts(ktg, P)],
                                        B_sb[:, dh, ts(q8 * nmm + nh, 512)],
                                        start=(dh == 0), stop=(dh == 1))
                            zi = kt * nqs + q8
                            nc.scalar.activation(
                                E_t[:, kt, ts(q8, qs)], sps[:], Exp,
                                scale=SCALE,
                                accum_out=Zp[:, zi:zi + 1])

                    # finalize Z, fold 1/Z into V
                    Z = z_pool.tile([P, kq], f32)
                    nc.vector.tensor_reduce(
                        Z[:], Zp[:].rearrange("p (k q) -> p k q", k=kq),
                        axis=mybir.AxisListType.X, op=mybir.AluOpType.add)
                    rz = z_pool.tile([P, kq], f32)
                    nc.vector.reciprocal(rz[:], Z[:])
                    Vp = vp_pool.tile([P, kq, D], bf16)
                    for kt in range(kq):
                        nc.vector.tensor_scalar_mul(
                            Vp[:, kt, :], V_sb[:, sub * kq + kt, :],
                            rz[:, kt:kt + 1])

                    # pass 2: outT_partial[j, q] = sum_k V'[k, j] * E[k, q]
                    for q8 in range(nqs):
                        for j in range(2):
                            for nh in range(nmm):
                                ops = psO.tile([P, 512], f32)
                                for kt in range(kq):
                                    nc.tensor.matmul(
                                        ops[:], Vp[:, kt, ts(j, P)],
                                        E_t[:, kt, ts(q8 * nmm + nh, 512)],
                                        start=(kt == 0), stop=(kt == kq - 1))
                                ost = o_pool.tile([P, 512], f32)
                                nc.vector.tensor_copy(ost[:], ops[:])
                                nc.sync.dma_start(
                                    out_part[sub, ts(j, P),
                                             ts(q8 * nmm + nh, 512)], ost[:])

    nc.compile()
    return nc


@functools.lru_cache(maxsize=1)
def _get_compiled():
    return _build_program()


def kernel(x, Wq, Wk, Wv):
    from concourse.bass_utils import run_bass_kernel_spmd

    nc = _get_compiled()

    x = np.ascontiguousarray(x, dtype=np.float32)
    in_maps = []
    for c in range(N_CORES):
        b = c // CORES_PER_BATCH
        k0 = (c % CORES_PER_BATCH) * CHUNK
        in_maps.append({
            "xb": x[b],
            "xk": np.ascontiguousarray(x[b, k0:k0 + CHUNK]),
            "wq": np.ascontiguousarray(Wq, dtype=np.float32),
            "wk": np.ascontiguousarray(Wk, dtype=np.float32),
            "wv": np.ascontiguousarray(Wv, dtype=np.float32),
        })

    res = run_bass_kernel_spmd(nc, in_maps, list(range(N_CORES)))
    global LAST_RESULTS, LAST_EXEC_TIME_NS
    LAST_RESULTS = res
    LAST_EXEC_TIME_NS = res.exec_time_ns

    out = np.empty((B, N, D), dtype=np.float32)
    for b in range(B):
        acc = np.zeros((D, N), dtype=np.float32)
        for c in range(b * CORES_PER_BATCH, (b + 1) * CORES_PER_BATCH):
            acc += res.results[c]["out_part"].sum(axis=0)
        out[b] = acc.T
    return out

